# revision 20
# baseline (speedup 1.0000x reference)
"""Trainium2 Bass kernel for nn_CausalTokenizer (MAE-style video tokenizer).

Sharding: 8 cores; core k handles batch b=k//2, frames t in {p, p+2, p+4, p+6}
(p = k%2, interleaved so causal time-attention work balances). Params are
replicated. Activations stay feature-on-partition (X^T) throughout; weights
are the stationary matmul operand, so no transposes are ever needed. The two
time-attention blocks exchange K^T/V^T within (even, odd) core pairs via an
AllGather collective.
"""
import os
import sys

sys.path.insert(0, "/opt/trn_rl_repo")

import numpy as np
import ml_dtypes

import concourse.bacc as bacc
import concourse.bass as bass
import concourse.mybir as mybir
from concourse.tile import TileContext
from concourse.bass_utils import run_bass_kernel_spmd

F32 = mybir.dt.float32
BF16 = mybir.dt.bfloat16
AF = mybir.ActivationFunctionType
ALU = mybir.AluOpType

# model dims
B, T, C, IMG, PP = 4, 8, 3, 224, 16
E, NH, NL, LD = 512, 8, 128, 128
G = IMG // PP
N = G * G            # 196
PD = C * PP * PP     # 768
MLP = 4 * E          # 2048
S = NL + N           # 324
EPS = 1e-5
NI = 4               # images per core
TOK = NI * S         # 1296
KT = E // 128        # 4
DH = E // NH         # 64

NCORES = 8
STAGE = os.environ.get("KERNEL_STAGE", "full")  # embed | enc | full

CHUNKS = [(0, 512), (512, 512), (1024, TOK - 1024)]
SKT = [(0, 128), (128, 128), (256, S - 256)]     # s_k tiles per image


def _bf(a):
    return np.ascontiguousarray(np.asarray(a, np.float32).astype(ml_dtypes.bfloat16))


def _f32(a):
    return np.ascontiguousarray(np.asarray(a, np.float32))


def _pack_w(W):
    """[K, M] -> [128, (M//128)*(K//128)*128], m-tile-major, contiguous DMA."""
    W = np.asarray(W, np.float32)
    K, M = W.shape
    kk, mts = K // 128, M // 128
    t = W.reshape(kk, 128, mts, 128).transpose(1, 2, 0, 3)
    return _bf(t.reshape(128, mts * kk * 128))


def _col(b):
    """[M] fp32 bias -> [128, M//128] (partition-major columns)."""
    b = _f32(b)
    return np.ascontiguousarray(b.reshape(-1, 128).T)


def build_program():
    nc = bacc.Bacc("TRN2", target_bir_lowering=False, debug=False,
                   num_devices=NCORES)
    g = {}
    g["xpT"] = nc.dram_tensor("xpT", [128, (PD // 128) * NI * N], BF16, kind="ExternalInput")
    g["maskrow"] = nc.dram_tensor("maskrow", [1, NI * N], F32, kind="ExternalInput")
    g["tm"] = nc.dram_tensor("tm", [8, NI], F32, kind="ExternalInput")
    g["hind"] = nc.dram_tensor("hind", [128, KT * NH], BF16, kind="ExternalInput")
    g["hsel"] = nc.dram_tensor("hsel", [8, KT * 128], BF16, kind="ExternalInput")

    g["wpe"] = nc.dram_tensor("wpe", [128, (PD // 128) * E], BF16, kind="ExternalInput")
    g["bpe"] = nc.dram_tensor("bpe", [128, KT], F32, kind="ExternalInput")
    g["mtok"] = nc.dram_tensor("mtok", [128, KT], F32, kind="ExternalInput")
    g["latT"] = nc.dram_tensor("latT", [E, NL], F32, kind="ExternalInput")
    g["decT"] = nc.dram_tensor("decT", [E, N], F32, kind="ExternalInput")
    g["wtl"] = nc.dram_tensor("wtl", [128, (E // 128) * LD], BF16, kind="ExternalInput")
    g["btl"] = nc.dram_tensor("btl", [128, 1], F32, kind="ExternalInput")
    g["wfl"] = nc.dram_tensor("wfl", [128, (LD // 128) * E], BF16, kind="ExternalInput")
    g["bfl"] = nc.dram_tensor("bfl", [128, KT], F32, kind="ExternalInput")
    g["wtp"] = nc.dram_tensor("wtp", [128, (E // 128) * PD], BF16, kind="ExternalInput")
    g["btp"] = nc.dram_tensor("btp", [128, PD // 128], F32, kind="ExternalInput")

    g["blk_w"] = []
    for bi in range(8):
        g["blk_w"].append({
            "wqkv": nc.dram_tensor(f"wqkv{bi}", [128, KT * 3 * E], BF16, kind="ExternalInput"),
            "bqkv": nc.dram_tensor(f"bqkv{bi}", [128, 12], F32, kind="ExternalInput"),
            "bqv": nc.dram_tensor(f"bqv{bi}", [1, E], F32, kind="ExternalInput"),
            "wproj": nc.dram_tensor(f"wproj{bi}", [128, KT * E], BF16, kind="ExternalInput"),
            "bproj": nc.dram_tensor(f"bproj{bi}", [128, KT], F32, kind="ExternalInput"),
            "wfc1": nc.dram_tensor(f"wfc1{bi}", [128, KT * MLP], BF16, kind="ExternalInput"),
            "bfc1": nc.dram_tensor(f"bfc1{bi}", [128, MLP // 128], F32, kind="ExternalInput"),
            "wfc2": nc.dram_tensor(f"wfc2{bi}", [128, (MLP // 128) * E], BF16, kind="ExternalInput"),
            "bfc2": nc.dram_tensor(f"bfc2{bi}", [128, KT], F32, kind="ExternalInput"),
            "ln1s": nc.dram_tensor(f"ln1s{bi}", [128, KT], F32, kind="ExternalInput"),
            "ln1b": nc.dram_tensor(f"ln1b{bi}", [128, KT], F32, kind="ExternalInput"),
            "ln2s": nc.dram_tensor(f"ln2s{bi}", [128, KT], F32, kind="ExternalInput"),
            "ln2b": nc.dram_tensor(f"ln2b{bi}", [128, KT], F32, kind="ExternalInput"),
        })

    g["patches_T"] = nc.dram_tensor("patches_T", [E, NI * N], F32, kind="ExternalOutput")
    g["z_T"] = nc.dram_tensor("z_T", [LD, NI * NL], F32, kind="ExternalOutput")
    g["rp_T"] = nc.dram_tensor("rp_T", [PD, NI * N], F32, kind="ExternalOutput")

    with TileContext(nc) as tc:
        _build_body(nc, tc, g)
    nc.compile()
    return nc


def _build_body(nc, tc, g):
    from contextlib import ExitStack
    ctx = ExitStack()
    pool = ctx.enter_context(tc.tile_pool(name="main", bufs=1))
    wpool = ctx.enter_context(tc.tile_pool(name="wts", bufs=1))
    spool = ctx.enter_context(tc.tile_pool(name="small", bufs=1))
    psum = ctx.enter_context(tc.tile_pool(name="ps", bufs=1, space="PSUM"))
    dram = ctx.enter_context(tc.tile_pool(name="dr", bufs=1, space="DRAM"))

    # persistent SBUF state (feature-major)
    XT = [pool.tile([128, NI, S], F32, name=f"xt{k}", tag=f"xt{k}") for k in range(KT)]
    hT = [pool.tile([128, NI, S], BF16, name=f"h{k}", tag=f"h{k}") for k in range(KT)]
    QT = [pool.tile([128, NI, S], BF16, name=f"qT{k}", tag=f"qT{k}") for k in range(KT)]
    KTa = [pool.tile([128, 2, NI, S], BF16, name=f"kT{k}", tag=f"kT{k}") for k in range(KT)]
    VTa = [pool.tile([128, 2, NI, S], BF16, name=f"vT{k}", tag=f"vT{k}") for k in range(KT)]
    AO = [pool.tile([128, NI, S], BF16, name=f"ao{k}", tag=f"ao{k}") for k in range(KT)]

    def flat(t):
        return t.rearrange("p i s -> p (i s)")

    cur = {}  # per-block weight handles

    # small consts
    tm_sb = spool.tile([8, NI], F32, tag="tmsb")
    nc.sync.dma_start(tm_sb[:], g["tm"][:])
    mrow = spool.tile([1, NI * N], F32, tag="rowtmp")
    nc.sync.dma_start(mrow[:], g["maskrow"][:])
    maskbc = pool.tile([128, NI * N], F32, tag="maskbc")
    nc.gpsimd.partition_broadcast(maskbc[:], mrow[:])
    ones_col = spool.tile([128, 1], BF16, tag="ones_col")
    nc.vector.memset(ones_col[:], 1.0)
    ones_row = spool.tile([1, 128], BF16, tag="ones_row")
    nc.vector.memset(ones_row[:], 1.0)

    hind_all = spool.tile([128, KT, NH], BF16, tag="hind_all")
    nc.sync.dma_start(hind_all[:], g["hind"].rearrange("p (k h) -> p k h", k=KT))
    hsel_all = spool.tile([8, KT, 128], BF16, tag="hsel_all")
    nc.sync.dma_start(hsel_all[:], g["hsel"].rearrange("p (k m) -> p k m", k=KT))
    HeadInd = [hind_all[:, k, :] for k in range(KT)]
    HeadSel = [hsel_all[:, k, :] for k in range(KT)]

    def load_cols(name, dram_t, ncols, tag=None, bufs=1):
        t = spool.tile([128, ncols], F32, name=name, tag=tag or name, bufs=bufs)
        nc.sync.dma_start(t[:], dram_t[:])
        return t

    bpe_sb = load_cols("bpe_sb", g["bpe"], KT)
    mtok_sb = load_cols("mtok_sb", g["mtok"], KT)
    btl_sb = load_cols("btl_sb", g["btl"], 1)
    bfl_sb = load_cols("bfl_sb", g["bfl"], KT)
    btp_sb = load_cols("btp_sb", g["btp"], PD // 128)

    def stream_w(dram_w, kk, tag="wms", bufs=4):
        # dram_w is host-packed [128, mts*kk*128]; each m-tile slice is
        # contiguous per partition.
        def w_fn(mt):
            t = wpool.tile([128, kk, 128], BF16, name="wst", tag=tag, bufs=bufs)
            nc.sync.dma_start(t[:], dram_w[:, mt * kk * 128:(mt + 1) * kk * 128]
                              .rearrange("p (k m) -> p k m", k=kk))
            return t
        return w_fn

    # ---------------- layernorm (chunk-wise, stats via ones-matmul) ----------
    def layernorm():
        for (n0, nw) in CHUNKS:
            xb = pool.tile([128, KT, 512], BF16, name="xb", tag="ln_xb", bufs=1)
            sq = pool.tile([128, KT, 512], BF16, name="sq", tag="ln_sq", bufs=1)
            for k in range(KT):
                xs = flat(XT[k])[:, n0:n0 + nw]
                nc.vector.tensor_copy(xb[:, k, :nw], xs)
                nc.vector.tensor_mul(sq[:, k, :nw], xs, xs)
            ps_s = psum.tile([1, 512], F32, name="lns", tag="ps_sm", bufs=2)
            ps_q = psum.tile([1, 512], F32, name="lnq", tag="ps_sm", bufs=2)
            for k in range(KT):
                nc.tensor.matmul(ps_s[:, :nw], ones_col[:], xb[:, k, :nw],
                                 start=(k == 0), stop=(k == KT - 1))
            for k in range(KT):
                nc.tensor.matmul(ps_q[:, :nw], ones_col[:], sq[:, k, :nw],
                                 start=(k == 0), stop=(k == KT - 1))
            nm = spool.tile([1, 512], F32, name="nm", tag="ln_nm", bufs=1)
            var = spool.tile([1, 512], F32, name="var", tag="ln_var", bufs=1)
            inv = spool.tile([1, 512], F32, name="inv", tag="ln_inv", bufs=1)
            nc.vector.tensor_scalar_mul(nm[:, :nw], ps_s[:, :nw], -1.0 / E)
            nc.vector.tensor_scalar_mul(var[:, :nw], ps_q[:, :nw], 1.0 / E)
            nc.vector.tensor_mul(inv[:, :nw], nm[:, :nw], nm[:, :nw])
            nc.vector.tensor_sub(var[:, :nw], var[:, :nw], inv[:, :nw])
            nc.vector.tensor_scalar_add(var[:, :nw], var[:, :nw], EPS)
            nc.vector.reciprocal(inv[:, :nw], var[:, :nw])
            nc.scalar.sqrt(var[:, :nw], inv[:, :nw])          # 1/std
            nc.vector.tensor_mul(nm[:, :nw], nm[:, :nw], var[:, :nw])  # -mu/std
            bci = pool.tile([128, 512], F32, name="bci", tag="ln_bci", bufs=2)
            bcn = pool.tile([128, 512], F32, name="bcn", tag="ln_bcn", bufs=1)
            nc.gpsimd.partition_broadcast(bci[:, :nw], var[:, :nw])
            nc.gpsimd.partition_broadcast(bcn[:, :nw], nm[:, :nw])
            for k in range(KT):
                t1 = pool.tile([128, 512], F32, name="t1", tag="ln_t1", bufs=1)
                nc.gpsimd.tensor_mul(t1[:, :nw], flat(XT[k])[:, n0:n0 + nw],
                                     bci[:, :nw])
                nc.vector.tensor_add(flat(hT[k])[:, n0:n0 + nw], t1[:, :nw],
                                     bcn[:, :nw])

    # ---------------- generic feature-major linear ----------------
    def linearT(w_fn, kk, m_tiles, rhs_fn, out_fn, n_slices=CHUNKS,
                m_order=None):
        for mt in (m_order if m_order is not None else range(m_tiles)):
            wt = w_fn(mt)
            for (n0, nw) in n_slices:
                ps = psum.tile([128, 512], F32, name="mmp", tag="ps_mm", bufs=3)
                for k in range(kk):
                    nc.tensor.matmul(ps[:, :nw], wt[:, k, :],
                                     rhs_fn(k, n0, nw),
                                     start=(k == 0), stop=(k == kk - 1))
                out_fn(mt, n0, nw, ps)

    def resid_add(mt, n0, nw, ps, bias_col):
        rt = pool.tile([128, 512], BF16, name="rt", tag="resid_t", bufs=2)
        nc.scalar.activation(rt[:, :nw], ps[:, :nw], AF.Identity,
                             bias=bias_col)
        xs = flat(XT[mt])[:, n0:n0 + nw]
        nc.gpsimd.tensor_add(xs, xs, rt[:, :nw])

    # ---------------- patch embed + masking ----------------
    img_slices = [(i * N, N) for i in range(NI)]
    xpT_sb = pool.tile([128, PD // 128, NI * N], BF16, name="xpT_sb", tag="bigstage")
    nc.sync.dma_start(xpT_sb[:], g["xpT"].rearrange("p (k m) -> p k m", k=PD // 128))
    wpe_fn = stream_w(g["wpe"], PD // 128, tag="wpes", bufs=2)
    for k in range(KT):
        for i in range(NI):
            nc.sync.dma_start(XT[k][:, i, 0:NL],
                              g["latT"][k * 128:(k + 1) * 128, :])

    def pe_out(mt, n0, nw, ps):
        i = n0 // N
        stg = pool.tile([128, N], F32, name="pstg", tag="stg_f32", bufs=3)
        nc.scalar.activation(stg[:], ps[:, :nw], AF.Identity,
                             bias=bpe_sb[:, mt:mt + 1])
        nc.sync.dma_start(g["patches_T"][mt * 128:(mt + 1) * 128, n0:n0 + nw],
                          stg[:])
        t2 = pool.tile([128, N], F32, name="t2", tag="stg_f32b", bufs=2)
        nc.vector.tensor_scalar_sub(t2[:], stg[:], mtok_sb[:, mt:mt + 1])
        nc.vector.tensor_mul(t2[:], t2[:], maskbc[:, n0:n0 + nw])
        nc.vector.tensor_scalar_add(XT[mt][:, i, NL:], t2[:],
                                    mtok_sb[:, mt:mt + 1])

    linearT(wpe_fn, PD // 128, KT,
            lambda k, n0, nw: xpT_sb[:, k, n0:n0 + nw],
            pe_out, n_slices=img_slices)

    if STAGE == "embed":
        ctx.close()
        return

    # ---------------- space attention ----------------
    def space_attention(mode):
        Vsb = {}
        for i in range(NI):
            for (t0, ts_) in SKT:
                v = pool.tile([128, NH, DH + 1], BF16, name="vsb", tag="vsb",
                              bufs=3)
                ps = psum.tile([128, 512], F32, name="vp", tag="ps_mm", bufs=3)
                for k in range(KT):
                    nc.tensor.matmul(ps[:ts_, :], hT[k][:, i, t0:t0 + ts_],
                                     cur["wv"][:, k].rearrange("p a b -> p (a b)"),
                                     start=(k == 0), stop=False)
                nc.tensor.matmul(ps[:ts_, :], ones_row[:, :ts_],
                                 cur["bqvb"][:], start=False, stop=True)
                nc.scalar.activation(
                    v[:ts_, :, 0:DH],
                    ps[:ts_, :].rearrange("p (h d) -> p h d", h=NH),
                    AF.Identity)
                nc.vector.memset(v[:ts_, :, DH:DH + 1], 1.0)
                Vsb[(i, t0)] = v
        for i in range(NI):
            for hg in range(2):
                drow = spool.tile([1, 4 * S], F32, name="drow", tag="drow",
                                  bufs=1)
                for hh in range(4):
                    h = 4 * hg + hh
                    kh, r0 = h // 2, 64 * (h % 2)
                    ex = {}
                    for (t0, ts_) in SKT:
                        ps = psum.tile([128, S], F32, name="scp", tag="ps_att",
                                       bufs=2)
                        nc.tensor.matmul(ps[:ts_, :],
                                         KTa[kh][r0:r0 + 64, 0, i, t0:t0 + ts_],
                                         QT[kh][r0:r0 + 64, i, :],
                                         start=True, stop=True)
                        e = pool.tile([128, S], BF16, name="exs", tag="exs",
                                      bufs=3)
                        if mode == "enc":
                            if t0 == 0:
                                nc.scalar.activation(e[:ts_, 0:NL],
                                                     ps[:ts_, 0:NL],
                                                     AF.Exp, scale=0.125)
                                nc.vector.memset(e[:ts_, NL:], 0.0)
                            else:
                                nc.scalar.activation(e[:ts_, :], ps[:ts_, :],
                                                     AF.Exp, scale=0.125)
                        else:
                            if t0 == 0:
                                nc.scalar.activation(e[:ts_, :], ps[:ts_, :],
                                                     AF.Exp, scale=0.125)
                            else:
                                nc.vector.memset(e[:ts_, 0:NL], 0.0)
                                nc.scalar.activation(e[:ts_, NL:],
                                                     ps[:ts_, NL:],
                                                     AF.Exp, scale=0.125)
                        ex[t0] = e
                    po = psum.tile([128, S], F32, name="avp", tag="ps_bc",
                                   bufs=1)
                    for j, (t0, ts_) in enumerate(SKT):
                        nc.tensor.matmul(po[0:DH + 1, :],
                                         Vsb[(i, t0)][:ts_, h, :],
                                         ex[t0][:ts_, :],
                                         start=(j == 0), stop=(j == 2))
                    nc.scalar.activation(drow[:, hh * S:(hh + 1) * S],
                                         po[DH:DH + 1, :], AF.Identity)
                    nc.scalar.activation(AO[kh][r0:r0 + 64, i, :],
                                         po[0:DH, :], AF.Identity)
                nc.vector.reciprocal(drow[:], drow[:])
                for hh in range(4):
                    h = 4 * hg + hh
                    kh, r0 = h // 2, 64 * (h % 2)
                    bcr = pool.tile([128, S], F32, name="bcr", tag="bcr", bufs=2)
                    nc.gpsimd.partition_broadcast(
                        bcr[:], drow[:, hh * S:(hh + 1) * S])
                    nc.vector.tensor_mul(AO[kh][r0:r0 + 64, i, :],
                                         AO[kh][r0:r0 + 64, i, :],
                                         bcr[r0:r0 + 64, :])

    # ---------------- time attention ----------------
    def time_attention():
        ccinK = dram.tile([E, TOK], BF16, name="ccinK", tag="ccinK")
        ccinV = dram.tile([E, TOK], BF16, name="ccinV", tag="ccinV")
        ccoutK = dram.tile([2 * E, TOK], BF16, name="ccoutK", tag="ccoutK")
        ccoutV = dram.tile([2 * E, TOK], BF16, name="ccoutV", tag="ccoutV")

        def qkv_out(mt, n0, nw, ps):
            if mt < KT:
                nc.scalar.activation(flat(QT[mt])[:, n0:n0 + nw], ps[:, :nw],
                                     AF.Identity,
                                     bias=cur["bqkv"][:, mt:mt + 1])
            else:
                stg = pool.tile([128, 512], BF16, name="kvstg", tag="kvstg",
                                bufs=1)
                nc.scalar.activation(stg[:, :nw], ps[:, :nw], AF.Identity,
                                     bias=cur["bqkv"][:, mt:mt + 1])
                cc = ccinK if mt < 2 * KT else ccinV
                nc.sync.dma_start(
                    cc[(mt % KT) * 128:(mt % KT + 1) * 128, n0:n0 + nw],
                    stg[:, :nw])

        linearT(cur["wqkv_fn"], KT, 12,
                lambda k, n0, nw: flat(hT[k])[:, n0:n0 + nw], qkv_out,
                m_order=[4, 5, 6, 7, 8, 9, 10, 11, 0, 1, 2, 3])
        nc.gpsimd.collective_compute(
            "AllGather", ALU.bypass, ins=[ccinK[:]], outs=[ccoutK[:]],
            replica_groups=[[0, 1], [2, 3], [4, 5], [6, 7]])
        nc.gpsimd.collective_compute(
            "AllGather", ALU.bypass, ins=[ccinV[:]], outs=[ccoutV[:]],
            replica_groups=[[0, 1], [2, 3], [4, 5], [6, 7]])
        for k in range(KT):
            for hf in range(2):
                base = hf * E
                nc.sync.dma_start(
                    KTa[k][:, hf, :, :],
                    ccoutK[base + k * 128:base + (k + 1) * 128, :]
                    .rearrange("p (i s) -> p i s", i=NI))
                nc.sync.dma_start(
                    VTa[k][:, hf, :, :],
                    ccoutV[base + k * 128:base + (k + 1) * 128, :]
                    .rearrange("p (i s) -> p i s", i=NI))
        # query image i (t = 2i+p); keys (j, hf) in superset 2j+hf <= 2i+1.
        # On even cores the (j==i, hf==1) pair is zeroed via the tm input.
        for i in range(NI):
            pairs = [(j, hf) for j in range(i + 1) for hf in range(2)]
            nt = len(pairs)
            SC = pool.tile([NH, 8, S], BF16, name="sct", tag="sct", bufs=1)
            for tk, (j, hf) in enumerate(pairs):
                pr = pool.tile([128, KT, S], BF16, name="prt", tag="prt", bufs=1)
                for k in range(KT):
                    nc.vector.tensor_mul(pr[:, k, :], QT[k][:, i, :],
                                         KTa[k][:, hf, j, :])
                pp = psum.tile([NH, S], F32, name="ppt", tag="ps_sm", bufs=2)
                for k in range(KT):
                    nc.tensor.matmul(pp[:], HeadInd[k], pr[:, k, :],
                                     start=(k == 0), stop=(k == KT - 1))
                nc.scalar.activation(SC[:, tk, :], pp[:], AF.Exp, scale=0.125)
                if j == i and hf == 1:
                    nc.vector.tensor_scalar_mul(SC[:, tk, :], SC[:, tk, :],
                                                tm_sb[:, i:i + 1])
            rd = spool.tile([NH, S], F32, name="rd", tag="rd", bufs=1)
            nc.vector.tensor_reduce(rd[:],
                                    SC[:, 0:nt, :].rearrange("p t s -> p s t"),
                                    axis=mybir.AxisListType.X, op=ALU.add)
            nc.vector.reciprocal(rd[:], rd[:])
            for tk in range(nt):
                nc.vector.tensor_mul(SC[:, tk, :], SC[:, tk, :], rd[:])
            for k in range(KT):
                acc = psum.tile([128, S], F32, name="acct", tag="ps_att", bufs=2)
                for tk, (j, hf) in enumerate(pairs):
                    pb = psum.tile([128, S], F32, name="pbt", tag="ps_bc", bufs=1)
                    nc.tensor.matmul(pb[:], HeadSel[k], SC[:, tk, :],
                                     start=True, stop=True)
                    if tk == 0:
                        nc.vector.tensor_mul(acc[:], pb[:], VTa[k][:, hf, j, :])
                    else:
                        tmpv = pool.tile([128, S], F32, name="tmpv", tag="tmpv",
                                         bufs=1)
                        nc.vector.tensor_mul(tmpv[:], pb[:], VTa[k][:, hf, j, :])
                        nc.vector.tensor_add(acc[:], acc[:], tmpv[:])
                nc.vector.tensor_copy(AO[k][:, i, :], acc[:])

    # ---------------- transformer block ----------------
    def block(bi, kind, mode):
        w = g["blk_w"][bi]
        cur["wqkv_fn"] = stream_w(w["wqkv"], KT, tag="wms", bufs=4)
        cur["bqkv"] = load_cols(f"bqkv_sb{bi}", w["bqkv"], 12, tag="c_bqkv", bufs=4)
        layernorm()
        if kind == "space":
            bvr = spool.tile([1, E], F32, name="bvr", tag="rowtmp", bufs=1)
            nc.sync.dma_start(bvr[:], w["bqv"][:])
            bqvb = spool.tile([1, E], BF16, name="bqvb", tag="bqvb", bufs=1)
            nc.vector.tensor_copy(bqvb[:], bvr[:])
            cur["bqvb"] = bqvb

            def qk_out(mt, n0, nw, ps):
                if mt < KT:
                    dst = flat(QT[mt])[:, n0:n0 + nw]
                else:
                    dst = flat(KTa[mt - KT][:, 0, :, :])[:, n0:n0 + nw]
                nc.scalar.activation(dst, ps[:, :nw], AF.Identity,
                                     bias=cur["bqkv"][:, mt:mt + 1])

            wv = wpool.tile([128, KT, KT, 128], BF16, name="wv", tag="wv",
                            bufs=2)
            for vmt in range(KT):
                nc.sync.dma_start(
                    wv[:, :, vmt, :],
                    w["wqkv"][:, (8 + vmt) * KT * 128:(9 + vmt) * KT * 128]
                    .rearrange("p (k m) -> p k m", k=KT))
            cur["wv"] = wv
            linearT(cur["wqkv_fn"], KT, 2 * KT,
                    lambda k, n0, nw: flat(hT[k])[:, n0:n0 + nw], qk_out)
            space_attention(mode)
        else:
            time_attention()
        bproj_sb = load_cols(f"bproj_sb{bi}", w["bproj"], KT, tag="c_b", bufs=4)
        linearT(stream_w(w["wproj"], KT, tag="wms", bufs=4), KT, KT,
                lambda k, n0, nw: flat(AO[k])[:, n0:n0 + nw],
                lambda mt, n0, nw, ps: resid_add(mt, n0, nw, ps,
                                                 bproj_sb[:, mt:mt + 1]))
        layernorm()
        bfc1_sb = load_cols(f"bfc1_sb{bi}", w["bfc1"], MLP // 128, tag="c_fc1", bufs=4)
        wfc2_fn = stream_w(w["wfc2"], MLP // 128, tag="w2s", bufs=2)
        bfc2_sb = load_cols(f"bfc2_sb{bi}", w["bfc2"], KT, tag="c_b", bufs=4)
        for (n0, nw) in CHUNKS:
            gt = pool.tile([128, MLP // 128, 512], BF16, name="gt", tag="gt",
                           bufs=1)
            for mt in range(MLP // 128):
                w1 = wpool.tile([128, KT, 128], BF16, name="w1", tag="w1s",
                                bufs=4)
                nc.sync.dma_start(
                    w1[:], w["wfc1"][:, mt * KT * 128:(mt + 1) * KT * 128]
                    .rearrange("p (k m) -> p k m", k=KT))
                ps = psum.tile([128, 512], F32, name="f1p", tag="ps_mm", bufs=3)
                for k in range(KT):
                    nc.tensor.matmul(ps[:, :nw], w1[:, k, :],
                                     flat(hT[k])[:, n0:n0 + nw],
                                     start=(k == 0), stop=(k == KT - 1))
                nc.scalar.activation(gt[:, mt, :nw], ps[:, :nw],
                                     AF.Gelu_apprx_tanh,
                                     bias=bfc1_sb[:, mt:mt + 1])
            for mo in range(KT):
                w2 = wfc2_fn(mo)
                ps = psum.tile([128, 512], F32, name="f2p", tag="ps_mm", bufs=3)
                for k in range(MLP // 128):
                    nc.tensor.matmul(ps[:, :nw], w2[:, k, :], gt[:, k, :nw],
                                     start=(k == 0), stop=(k == MLP // 128 - 1))
                resid_add(mo, n0, nw, ps, bfc2_sb[:, mo:mo + 1])

    # encoder
    for bi in range(3):
        block(bi, "space", "enc")
    block(3, "time", None)

    # to_latent + tanh -> z
    zin = pool.tile([128, KT, NI, NL], BF16, name="zin", tag="bigstage")
    for k in range(KT):
        nc.vector.tensor_copy(zin[:, k, :, :], XT[k][:, :, 0:NL])
    wtl_sb = stream_w(g["wtl"], KT, tag="wms", bufs=4)(0)
    zps = psum.tile([128, 512], F32, name="zps", tag="ps_mm", bufs=3)
    for k in range(KT):
        nc.tensor.matmul(zps[:], wtl_sb[:, k, :],
                         zin[:, k].rearrange("p i s -> p (i s)"),
                         start=(k == 0), stop=(k == KT - 1))
    zT_sb = pool.tile([128, NI * NL], F32, name="zT_sb", tag="zT_sb")
    nc.scalar.activation(zT_sb[:], zps[:], AF.Tanh, bias=btl_sb[:, 0:1])
    nc.sync.dma_start(g["z_T"][:], zT_sb[:])

    if STAGE == "enc":
        ctx.close()
        return

    # from_latent -> XT latent cols; decoder tokens -> patch cols
    zb = pool.tile([128, NI * NL], BF16, name="zb", tag="zb")
    nc.vector.tensor_copy(zb[:], zT_sb[:])
    wfl_fn = stream_w(g["wfl"], 1, tag="wms", bufs=4)
    for mt in range(KT):
        wfl_t = wfl_fn(mt)
        ps = psum.tile([128, 512], F32, name="flp", tag="ps_mm", bufs=3)
        nc.tensor.matmul(ps[:], wfl_t[:, 0, :], zb[:],
                         start=True, stop=True)
        nc.scalar.activation(
            XT[mt][:, :, 0:NL], ps[:].rearrange("p (i s) -> p i s", i=NI),
            AF.Identity, bias=bfl_sb[:, mt:mt + 1])
    for k in range(KT):
        for i in range(NI):
            nc.sync.dma_start(XT[k][:, i, NL:],
                              g["decT"][k * 128:(k + 1) * 128, :])

    # decoder
    for bi in range(3):
        block(4 + bi, "space", "dec")
    block(7, "time", None)

    # to_pixels on patch tokens
    hP = pool.tile([128, KT, NI, N], BF16, name="hP", tag="bigstage")
    for k in range(KT):
        nc.vector.tensor_copy(hP[:, k, :, :], XT[k][:, :, NL:])
    wtp_fn = stream_w(g["wtp"], KT, tag="wms", bufs=4)

    def tp_out(mt, n0, nw, ps):
        stg = pool.tile([128, N], F32, name="tpst", tag="stg_f32", bufs=3)
        nc.scalar.activation(stg[:], ps[:, :nw], AF.Identity,
                             bias=btp_sb[:, mt:mt + 1])
        nc.sync.dma_start(g["rp_T"][mt * 128:(mt + 1) * 128, n0:n0 + nw],
                          stg[:])

    linearT(wtp_fn, KT, PD // 128,
            lambda k, n0, nw: hP[:, k, n0 // N, :], tp_out,
            n_slices=img_slices)
    ctx.close()


# ---------------- host side ----------------
_prog_cache = {}


def _get_prog():
    if "nc" not in _prog_cache:
        _prog_cache["nc"] = build_program()
    return _prog_cache["nc"]


def _host_mask(noise, mask_ratios):
    ids_shuffle = np.argsort(noise, axis=2, kind="stable")
    ids_unshuffle = np.argsort(ids_shuffle, axis=2, kind="stable")
    len_keep = (np.float32(N) * (np.float32(1.0) -
                                 mask_ratios.astype(np.float32) *
                                 np.float32(0.9))).astype(np.int32)
    vis = (np.arange(N)[None, None, :] < len_keep[:, None, None]).astype(np.float32)
    return np.take_along_axis(np.broadcast_to(vis, (B, T, N)).copy(),
                              ids_unshuffle, axis=2)


def kernel(x, noise, mask_ratios, params):
    x = np.asarray(x, np.float32)
    noise = np.asarray(noise, np.float32)
    mask_ratios = np.asarray(mask_ratios, np.float32)
    p = params

    mask = _host_mask(noise, mask_ratios)
    xp = x.reshape(B, T, C, G, PP, G, PP).transpose(0, 1, 3, 5, 4, 6, 2) \
          .reshape(B, T, N, PD)

    hind = np.zeros((128, KT, NH), np.float32)
    hsel = np.zeros((8, KT, 128), np.float32)
    for k in range(KT):
        hind[0:64, k, 2 * k] = 1.0
        hind[64:128, k, 2 * k + 1] = 1.0
        hsel[2 * k, k, 0:64] = 1.0
        hsel[2 * k + 1, k, 64:128] = 1.0
    shared = {
        "hind": _bf(hind.reshape(128, KT * NH)),
        "hsel": _bf(hsel.reshape(8, KT * 128)),
        "wpe": _pack_w(p["patch_embed"]["w"]), "bpe": _col(p["patch_embed"]["b"]),
        "mtok": _col(p["mask_token"]),
        "latT": _f32(np.asarray(p["latent_tokens"]).T),
        "decT": _f32(np.asarray(p["decoder_tokens"]).T),
        "wtl": _pack_w(p["to_latent"]["w"]), "btl": _col(p["to_latent"]["b"]),
        "wfl": _pack_w(p["from_latent"]["w"]), "bfl": _col(p["from_latent"]["b"]),
        "wtp": _pack_w(p["to_pixels"]["w"]), "btp": _col(p["to_pixels"]["b"]),
    }
    allblk = list(p["enc_blocks"]) + list(p["dec_blocks"])
    for bi in range(8):
        blk = allblk[bi]
        l1s, l1b = _f32(blk["ln1_s"]), _f32(blk["ln1_b"])
        l2s, l2b = _f32(blk["ln2_s"]), _f32(blk["ln2_b"])
        wqkv = _f32(blk["qkv"]["w"])
        wqkv_s = l1s[:, None] * wqkv
        bqkv_f = _f32(blk["qkv"]["b"]) + l1b @ wqkv
        wfc1 = _f32(blk["fc1"]["w"])
        wfc1_s = l2s[:, None] * wfc1
        bfc1_f = _f32(blk["fc1"]["b"]) + l2b @ wfc1
        shared.update({
            f"wqkv{bi}": _pack_w(wqkv_s), f"bqkv{bi}": _col(bqkv_f),
            f"bqv{bi}": bqkv_f[2 * E:3 * E].reshape(1, E).copy(),
            f"wproj{bi}": _pack_w(blk["proj"]["w"]), f"bproj{bi}": _col(blk["proj"]["b"]),
            f"wfc1{bi}": _pack_w(wfc1_s), f"bfc1{bi}": _col(bfc1_f),
            f"wfc2{bi}": _pack_w(blk["fc2"]["w"]), f"bfc2{bi}": _col(blk["fc2"]["b"]),
            f"ln1s{bi}": _col(l1s), f"ln1b{bi}": _col(l1b),
            f"ln2s{bi}": _col(l2s), f"ln2b{bi}": _col(l2b),
        })

    core_t = [[2 * i + (k % 2) for i in range(NI)] for k in range(NCORES)]
    in_maps = []
    for k in range(NCORES):
        b, pbit = k // 2, k % 2
        ts = core_t[k]
        xpc = xp[b, ts]                       # [NI, N, PD]
        m = dict(shared)
        xpt = xpc.transpose(2, 0, 1).reshape(PD // 128, 128, NI * N)
        m["xpT"] = _bf(xpt.transpose(1, 0, 2).reshape(128, (PD // 128) * NI * N))
        m["maskrow"] = _f32(mask[b, ts].reshape(1, NI * N))
        m["tm"] = np.full((8, NI), 0.0 if pbit == 0 else 1.0, np.float32)
        in_maps.append(m)

    nc = _get_prog()
    res = run_bass_kernel_spmd(nc, in_maps, list(range(NCORES)))

    patches = np.zeros((B, T, N, E), np.float32)
    z = np.zeros((B, T, NL, LD), np.float32)
    rp = np.zeros((B, T, N, PD), np.float32)
    for k in range(NCORES):
        b, ts = k // 2, core_t[k]
        r = res.results[k]
        patches[b, ts] = r["patches_T"].reshape(E, NI, N).transpose(1, 2, 0)
        z[b, ts] = r["z_T"].reshape(LD, NI, NL).transpose(1, 2, 0)
        rp[b, ts] = r["rp_T"].reshape(PD, NI, N).transpose(1, 2, 0)

    recon = rp.reshape(B, T, G, G, PP, PP, C).transpose(0, 1, 6, 2, 4, 3, 5) \
              .reshape(B, T, C, IMG, IMG)
    return z, recon, mask.astype(np.float32), patches


# revision 21
# speedup vs baseline: 1.4129x; 1.4129x over previous
"""Trainium2 Bass kernel for nn_CausalTokenizer (MAE-style video tokenizer).

Sharding: 8 cores; core k handles batch b=k//2, frames t in {p, p+2, p+4, p+6}
(p = k%2, interleaved so causal time-attention work balances). Params are
replicated. Activations stay feature-on-partition (X^T) throughout; weights
are the stationary matmul operand, so no transposes are ever needed. The two
time-attention blocks exchange K^T/V^T within (even, odd) core pairs via an
AllGather collective.
"""
import os
import sys

sys.path.insert(0, "/opt/trn_rl_repo")

import numpy as np
import ml_dtypes

import concourse.bacc as bacc
import concourse.bass as bass
import concourse.mybir as mybir
from concourse.tile import TileContext
from concourse.bass_utils import run_bass_kernel_spmd

F32 = mybir.dt.float32
BF16 = mybir.dt.bfloat16
AF = mybir.ActivationFunctionType
ALU = mybir.AluOpType

# model dims
B, T, C, IMG, PP = 4, 8, 3, 224, 16
E, NH, NL, LD = 512, 8, 128, 128
G = IMG // PP
N = G * G            # 196
PD = C * PP * PP     # 768
MLP = 4 * E          # 2048
S = NL + N           # 324
EPS = 1e-5
NI = 4               # images per core
TOK = NI * S         # 1296
KT = E // 128        # 4
DH = E // NH         # 64

NCORES = 8
STAGE = os.environ.get("KERNEL_STAGE", "full")  # embed | enc | full

CHUNKS = [(0, 512), (512, 512), (1024, TOK - 1024)]
SKT = [(0, 128), (128, 128), (256, S - 256)]     # s_k tiles per image


def _bf(a):
    return np.ascontiguousarray(np.asarray(a, np.float32).astype(ml_dtypes.bfloat16))


def _f32(a):
    return np.ascontiguousarray(np.asarray(a, np.float32))


def _pack_w(W):
    """[K, M] -> [128, (M//128)*(K//128)*128], m-tile-major, contiguous DMA."""
    W = np.asarray(W, np.float32)
    K, M = W.shape
    kk, mts = K // 128, M // 128
    t = W.reshape(kk, 128, mts, 128).transpose(1, 2, 0, 3)
    return _bf(t.reshape(128, mts * kk * 128))


def _col(b):
    """[M] fp32 bias -> [128, M//128] (partition-major columns)."""
    b = _f32(b)
    return np.ascontiguousarray(b.reshape(-1, 128).T)


def build_program():
    nc = bacc.Bacc("TRN2", target_bir_lowering=False, debug=False,
                   num_devices=NCORES)
    g = {}
    g["xpT"] = nc.dram_tensor("xpT", [128, (PD // 128) * NI * N], BF16, kind="ExternalInput")
    g["maskrow"] = nc.dram_tensor("maskrow", [1, NI * N], F32, kind="ExternalInput")
    g["tm"] = nc.dram_tensor("tm", [8, NI], F32, kind="ExternalInput")
    g["hind"] = nc.dram_tensor("hind", [128, KT * NH], BF16, kind="ExternalInput")
    g["hsel"] = nc.dram_tensor("hsel", [8, KT * 128], BF16, kind="ExternalInput")

    g["wpe"] = nc.dram_tensor("wpe", [128, (PD // 128) * E], BF16, kind="ExternalInput")
    g["bpe"] = nc.dram_tensor("bpe", [128, KT], F32, kind="ExternalInput")
    g["mtok"] = nc.dram_tensor("mtok", [128, KT], F32, kind="ExternalInput")
    g["latT"] = nc.dram_tensor("latT", [E, NL], F32, kind="ExternalInput")
    g["decT"] = nc.dram_tensor("decT", [E, N], F32, kind="ExternalInput")
    g["wtl"] = nc.dram_tensor("wtl", [128, (E // 128) * LD], BF16, kind="ExternalInput")
    g["btl"] = nc.dram_tensor("btl", [128, 1], F32, kind="ExternalInput")
    g["wfl"] = nc.dram_tensor("wfl", [128, (LD // 128) * E], BF16, kind="ExternalInput")
    g["bfl"] = nc.dram_tensor("bfl", [128, KT], F32, kind="ExternalInput")
    g["wtp"] = nc.dram_tensor("wtp", [128, (E // 128) * PD], BF16, kind="ExternalInput")
    g["btp"] = nc.dram_tensor("btp", [128, PD // 128], F32, kind="ExternalInput")

    g["blk_w"] = []
    for bi in range(8):
        g["blk_w"].append({
            "wqkv": nc.dram_tensor(f"wqkv{bi}", [128, KT * 3 * E], BF16, kind="ExternalInput"),
            "bqkv": nc.dram_tensor(f"bqkv{bi}", [128, 12], F32, kind="ExternalInput"),
            "bqv": nc.dram_tensor(f"bqv{bi}", [1, E], F32, kind="ExternalInput"),
            "wproj": nc.dram_tensor(f"wproj{bi}", [128, KT * E], BF16, kind="ExternalInput"),
            "bproj": nc.dram_tensor(f"bproj{bi}", [128, KT], F32, kind="ExternalInput"),
            "wfc1": nc.dram_tensor(f"wfc1{bi}", [128, KT * MLP], BF16, kind="ExternalInput"),
            "bfc1": nc.dram_tensor(f"bfc1{bi}", [128, MLP // 128], F32, kind="ExternalInput"),
            "wfc2": nc.dram_tensor(f"wfc2{bi}", [128, (MLP // 128) * E], BF16, kind="ExternalInput"),
            "bfc2": nc.dram_tensor(f"bfc2{bi}", [128, KT], F32, kind="ExternalInput"),
            "ln1s": nc.dram_tensor(f"ln1s{bi}", [128, KT], F32, kind="ExternalInput"),
            "ln1b": nc.dram_tensor(f"ln1b{bi}", [128, KT], F32, kind="ExternalInput"),
            "ln2s": nc.dram_tensor(f"ln2s{bi}", [128, KT], F32, kind="ExternalInput"),
            "ln2b": nc.dram_tensor(f"ln2b{bi}", [128, KT], F32, kind="ExternalInput"),
        })

    g["patches_T"] = nc.dram_tensor("patches_T", [E, NI * N], F32, kind="ExternalOutput")
    g["z_T"] = nc.dram_tensor("z_T", [LD, NI * NL], F32, kind="ExternalOutput")
    g["rp_T"] = nc.dram_tensor("rp_T", [PD, NI * N], F32, kind="ExternalOutput")

    with TileContext(nc) as tc:
        _build_body(nc, tc, g)
    nc.compile()
    return nc


def _build_body(nc, tc, g):
    from contextlib import ExitStack
    ctx = ExitStack()
    pool = ctx.enter_context(tc.tile_pool(name="main", bufs=1))
    wpool = ctx.enter_context(tc.tile_pool(name="wts", bufs=1))
    spool = ctx.enter_context(tc.tile_pool(name="small", bufs=1))
    psum = ctx.enter_context(tc.tile_pool(name="ps", bufs=1, space="PSUM"))
    dram = ctx.enter_context(tc.tile_pool(name="dr", bufs=1, space="DRAM"))

    # persistent SBUF state (feature-major)
    XT = [pool.tile([128, NI, S], F32, name=f"xt{k}", tag=f"xt{k}") for k in range(KT)]
    hT = [pool.tile([128, NI, S], BF16, name=f"h{k}", tag=f"h{k}") for k in range(KT)]
    QT = [pool.tile([128, NI, S], BF16, name=f"qT{k}", tag=f"qT{k}") for k in range(KT)]
    KTa = [pool.tile([128, 2, NI, S], BF16, name=f"kT{k}", tag=f"kT{k}") for k in range(KT)]
    VTa = [pool.tile([128, 2, NI, S], BF16, name=f"vT{k}", tag=f"vT{k}") for k in range(KT)]
    AO = [pool.tile([128, NI, S], BF16, name=f"ao{k}", tag=f"ao{k}") for k in range(KT)]

    def flat(t):
        return t.rearrange("p i s -> p (i s)")

    cur = {}  # per-block weight handles

    # small consts
    tm_sb = spool.tile([8, NI], F32, tag="tmsb")
    nc.sync.dma_start(tm_sb[:], g["tm"][:])
    mrow = spool.tile([1, NI * N], F32, tag="rowtmp")
    nc.sync.dma_start(mrow[:], g["maskrow"][:])
    maskbc = pool.tile([128, NI * N], F32, tag="maskbc")
    nc.gpsimd.partition_broadcast(maskbc[:], mrow[:])
    ones_col = spool.tile([128, 1], BF16, tag="ones_col")
    nc.vector.memset(ones_col[:], 1.0)
    ones_row = spool.tile([1, 128], BF16, tag="ones_row")
    nc.vector.memset(ones_row[:], 1.0)

    hind_all = spool.tile([128, KT, NH], BF16, tag="hind_all")
    nc.sync.dma_start(hind_all[:], g["hind"].rearrange("p (k h) -> p k h", k=KT))
    hsel_all = spool.tile([8, KT, 128], BF16, tag="hsel_all")
    nc.sync.dma_start(hsel_all[:], g["hsel"].rearrange("p (k m) -> p k m", k=KT))
    HeadInd = [hind_all[:, k, :] for k in range(KT)]
    HeadSel = [hsel_all[:, k, :] for k in range(KT)]

    def load_cols(name, dram_t, ncols, tag=None, bufs=1):
        t = spool.tile([128, ncols], F32, name=name, tag=tag or name, bufs=bufs)
        nc.sync.dma_start(t[:], dram_t[:])
        return t

    bpe_sb = load_cols("bpe_sb", g["bpe"], KT)
    mtok_sb = load_cols("mtok_sb", g["mtok"], KT)
    btl_sb = load_cols("btl_sb", g["btl"], 1)
    bfl_sb = load_cols("bfl_sb", g["bfl"], KT)
    btp_sb = load_cols("btp_sb", g["btp"], PD // 128)

    def stream_w(dram_w, kk, tag="wms", bufs=4):
        # dram_w is host-packed [128, mts*kk*128]; each m-tile slice is
        # contiguous per partition.
        def w_fn(mt):
            t = wpool.tile([128, kk, 128], BF16, name="wst", tag=tag, bufs=bufs)
            nc.sync.dma_start(t[:], dram_w[:, mt * kk * 128:(mt + 1) * kk * 128]
                              .rearrange("p (k m) -> p k m", k=kk))
            return t
        return w_fn

    # ---------------- layernorm (chunk-wise, stats via ones-matmul) ----------
    def layernorm():
        for (n0, nw) in CHUNKS:
            xb = pool.tile([128, KT, 512], BF16, name="xb", tag="ln_xb", bufs=1)
            sq = pool.tile([128, KT, 512], BF16, name="sq", tag="ln_sq", bufs=1)
            for k in range(KT):
                xs = flat(XT[k])[:, n0:n0 + nw]
                nc.vector.tensor_copy(xb[:, k, :nw], xs)
                nc.vector.tensor_mul(sq[:, k, :nw], xs, xs)
            ps_s = psum.tile([1, 512], F32, name="lns", tag="ps_sm", bufs=2)
            ps_q = psum.tile([1, 512], F32, name="lnq", tag="ps_sm", bufs=2)
            for k in range(KT):
                nc.tensor.matmul(ps_s[:, :nw], ones_col[:], xb[:, k, :nw],
                                 start=(k == 0), stop=(k == KT - 1))
            for k in range(KT):
                nc.tensor.matmul(ps_q[:, :nw], ones_col[:], sq[:, k, :nw],
                                 start=(k == 0), stop=(k == KT - 1))
            nm = spool.tile([1, 512], F32, name="nm", tag="ln_nm", bufs=1)
            var = spool.tile([1, 512], F32, name="var", tag="ln_var", bufs=1)
            inv = spool.tile([1, 512], F32, name="inv", tag="ln_inv", bufs=1)
            nc.vector.tensor_scalar_mul(nm[:, :nw], ps_s[:, :nw], -1.0 / E)
            nc.vector.tensor_scalar_mul(var[:, :nw], ps_q[:, :nw], 1.0 / E)
            nc.vector.tensor_mul(inv[:, :nw], nm[:, :nw], nm[:, :nw])
            nc.vector.tensor_sub(var[:, :nw], var[:, :nw], inv[:, :nw])
            nc.vector.tensor_scalar_add(var[:, :nw], var[:, :nw], EPS)
            nc.scalar.activation(inv[:, :nw], var[:, :nw], AF.Ln)
            nc.scalar.activation(var[:, :nw], inv[:, :nw], AF.Exp,
                                 scale=-0.5)                  # 1/std
            nc.vector.tensor_mul(nm[:, :nw], nm[:, :nw], var[:, :nw])  # -mu/std
            bci = pool.tile([128, 512], F32, name="bci", tag="ln_bci", bufs=2)
            bcn = pool.tile([128, 512], F32, name="bcn", tag="ln_bcn", bufs=1)
            nc.gpsimd.partition_broadcast(bci[:, :nw], var[:, :nw])
            nc.gpsimd.partition_broadcast(bcn[:, :nw], nm[:, :nw])
            for k in range(KT):
                t1 = pool.tile([128, 512], F32, name="t1", tag="ln_t1", bufs=1)
                nc.vector.tensor_mul(t1[:, :nw], flat(XT[k])[:, n0:n0 + nw],
                                     bci[:, :nw])
                nc.vector.tensor_add(flat(hT[k])[:, n0:n0 + nw], t1[:, :nw],
                                     bcn[:, :nw])

    # ---------------- generic feature-major linear ----------------
    def linearT(w_fn, kk, m_tiles, rhs_fn, out_fn, n_slices=CHUNKS,
                m_order=None):
        for mt in (m_order if m_order is not None else range(m_tiles)):
            wt = w_fn(mt)
            for (n0, nw) in n_slices:
                ps = psum.tile([128, 512], F32, name="mmp", tag="ps_mm", bufs=3)
                for k in range(kk):
                    nc.tensor.matmul(ps[:, :nw], wt[:, k, :],
                                     rhs_fn(k, n0, nw),
                                     start=(k == 0), stop=(k == kk - 1))
                out_fn(mt, n0, nw, ps)

    def resid_add(mt, n0, nw, ps, bias_col):
        rt = pool.tile([128, 512], BF16, name="rt", tag="resid_t", bufs=2)
        nc.scalar.activation(rt[:, :nw], ps[:, :nw], AF.Identity,
                             bias=bias_col)
        xs = flat(XT[mt])[:, n0:n0 + nw]
        nc.vector.tensor_add(xs, xs, rt[:, :nw])

    # ---------------- patch embed + masking ----------------
    img_slices = [(i * N, N) for i in range(NI)]
    xpT_sb = pool.tile([128, PD // 128, NI * N], BF16, name="xpT_sb", tag="bigstage")
    nc.sync.dma_start(xpT_sb[:], g["xpT"].rearrange("p (k m) -> p k m", k=PD // 128))
    wpe_fn = stream_w(g["wpe"], PD // 128, tag="wpes", bufs=2)
    for k in range(KT):
        for i in range(NI):
            nc.sync.dma_start(XT[k][:, i, 0:NL],
                              g["latT"][k * 128:(k + 1) * 128, :])

    def pe_out(mt, n0, nw, ps):
        i = n0 // N
        stg = pool.tile([128, N], F32, name="pstg", tag="stg_f32", bufs=3)
        nc.scalar.activation(stg[:], ps[:, :nw], AF.Identity,
                             bias=bpe_sb[:, mt:mt + 1])
        nc.sync.dma_start(g["patches_T"][mt * 128:(mt + 1) * 128, n0:n0 + nw],
                          stg[:])
        t2 = pool.tile([128, N], F32, name="t2", tag="stg_f32b", bufs=2)
        nc.vector.tensor_scalar_sub(t2[:], stg[:], mtok_sb[:, mt:mt + 1])
        nc.vector.tensor_mul(t2[:], t2[:], maskbc[:, n0:n0 + nw])
        nc.vector.tensor_scalar_add(XT[mt][:, i, NL:], t2[:],
                                    mtok_sb[:, mt:mt + 1])

    linearT(wpe_fn, PD // 128, KT,
            lambda k, n0, nw: xpT_sb[:, k, n0:n0 + nw],
            pe_out, n_slices=img_slices)

    if STAGE == "embed":
        ctx.close()
        return

    # ---------------- space attention ----------------
    def space_attention(mode):
        Vsb = {}
        for i in range(NI):
            for (t0, ts_) in SKT:
                v = pool.tile([128, NH, DH + 1], BF16, name="vsb", tag="vsb",
                              bufs=3)
                ps = psum.tile([128, 512], F32, name="vp", tag="ps_mm", bufs=3)
                for k in range(KT):
                    nc.tensor.matmul(ps[:ts_, :], hT[k][:, i, t0:t0 + ts_],
                                     cur["wv"][:, k].rearrange("p a b -> p (a b)"),
                                     start=(k == 0), stop=False)
                nc.tensor.matmul(ps[:ts_, :], ones_row[:, :ts_],
                                 cur["bqvb"][:], start=False, stop=True)
                nc.scalar.activation(
                    v[:ts_, :, 0:DH],
                    ps[:ts_, :].rearrange("p (h d) -> p h d", h=NH),
                    AF.Identity)
                nc.vector.memset(v[:ts_, :, DH:DH + 1], 1.0)
                Vsb[(i, t0)] = v
        for i in range(NI):
            for hg in range(2):
                drow = spool.tile([1, 4 * S], F32, name="drow", tag="drow",
                                  bufs=1)
                for hh in range(4):
                    h = 4 * hg + hh
                    kh, r0 = h // 2, 64 * (h % 2)
                    ex = {}
                    for (t0, ts_) in SKT:
                        ps = psum.tile([128, S], F32, name="scp", tag="ps_att",
                                       bufs=2)
                        nc.tensor.matmul(ps[:ts_, :],
                                         KTa[kh][r0:r0 + 64, 0, i, t0:t0 + ts_],
                                         QT[kh][r0:r0 + 64, i, :],
                                         start=True, stop=True)
                        e = pool.tile([128, S], BF16, name="exs", tag="exs",
                                      bufs=3)
                        if mode == "enc":
                            if t0 == 0:
                                nc.scalar.activation(e[:ts_, 0:NL],
                                                     ps[:ts_, 0:NL],
                                                     AF.Exp, scale=0.125)
                                nc.vector.memset(e[:ts_, NL:], 0.0)
                            else:
                                nc.scalar.activation(e[:ts_, :], ps[:ts_, :],
                                                     AF.Exp, scale=0.125)
                        else:
                            if t0 == 0:
                                nc.scalar.activation(e[:ts_, :], ps[:ts_, :],
                                                     AF.Exp, scale=0.125)
                            else:
                                nc.vector.memset(e[:ts_, 0:NL], 0.0)
                                nc.scalar.activation(e[:ts_, NL:],
                                                     ps[:ts_, NL:],
                                                     AF.Exp, scale=0.125)
                        ex[t0] = e
                    po = psum.tile([128, S], F32, name="avp", tag="ps_bc",
                                   bufs=1)
                    for j, (t0, ts_) in enumerate(SKT):
                        nc.tensor.matmul(po[0:DH + 1, :],
                                         Vsb[(i, t0)][:ts_, h, :],
                                         ex[t0][:ts_, :],
                                         start=(j == 0), stop=(j == 2))
                    nc.scalar.activation(drow[:, hh * S:(hh + 1) * S],
                                         po[DH:DH + 1, :], AF.Identity)
                    nc.scalar.activation(AO[kh][r0:r0 + 64, i, :],
                                         po[0:DH, :], AF.Identity)
                nc.scalar.activation(drow[:], drow[:], AF.Ln)
                nc.scalar.activation(drow[:], drow[:], AF.Exp, scale=-1.0)
                for hh in range(4):
                    h = 4 * hg + hh
                    kh, r0 = h // 2, 64 * (h % 2)
                    bcr = pool.tile([128, S], F32, name="bcr", tag="bcr", bufs=2)
                    nc.gpsimd.partition_broadcast(
                        bcr[:], drow[:, hh * S:(hh + 1) * S])
                    nc.vector.tensor_mul(AO[kh][r0:r0 + 64, i, :],
                                         AO[kh][r0:r0 + 64, i, :],
                                         bcr[r0:r0 + 64, :])

    # ---------------- time attention ----------------
    def time_attention():
        ccinK = dram.tile([E, TOK], BF16, name="ccinK", tag="ccinK")
        ccinV = dram.tile([E, TOK], BF16, name="ccinV", tag="ccinV")
        ccoutK = dram.tile([2 * E, TOK], BF16, name="ccoutK", tag="ccoutK")
        ccoutV = dram.tile([2 * E, TOK], BF16, name="ccoutV", tag="ccoutV")

        def qkv_out(mt, n0, nw, ps):
            if mt < KT:
                nc.scalar.activation(flat(QT[mt])[:, n0:n0 + nw], ps[:, :nw],
                                     AF.Identity,
                                     bias=cur["bqkv"][:, mt:mt + 1])
            else:
                stg = pool.tile([128, 512], BF16, name="kvstg", tag="kvstg",
                                bufs=1)
                nc.scalar.activation(stg[:, :nw], ps[:, :nw], AF.Identity,
                                     bias=cur["bqkv"][:, mt:mt + 1])
                cc = ccinK if mt < 2 * KT else ccinV
                nc.sync.dma_start(
                    cc[(mt % KT) * 128:(mt % KT + 1) * 128, n0:n0 + nw],
                    stg[:, :nw])

        linearT(cur["wqkv_fn"], KT, 12,
                lambda k, n0, nw: flat(hT[k])[:, n0:n0 + nw], qkv_out,
                m_order=[4, 5, 6, 7, 8, 9, 10, 11, 0, 1, 2, 3])
        nc.gpsimd.collective_compute(
            "AllGather", ALU.bypass, ins=[ccinK[:]], outs=[ccoutK[:]],
            replica_groups=[[0, 1], [2, 3], [4, 5], [6, 7]])
        nc.gpsimd.collective_compute(
            "AllGather", ALU.bypass, ins=[ccinV[:]], outs=[ccoutV[:]],
            replica_groups=[[0, 1], [2, 3], [4, 5], [6, 7]])
        for k in range(KT):
            for hf in range(2):
                base = hf * E
                nc.sync.dma_start(
                    KTa[k][:, hf, :, :],
                    ccoutK[base + k * 128:base + (k + 1) * 128, :]
                    .rearrange("p (i s) -> p i s", i=NI))
                nc.sync.dma_start(
                    VTa[k][:, hf, :, :],
                    ccoutV[base + k * 128:base + (k + 1) * 128, :]
                    .rearrange("p (i s) -> p i s", i=NI))
        # query image i (t = 2i+p); keys (j, hf) in superset 2j+hf <= 2i+1.
        # On even cores the (j==i, hf==1) pair is zeroed via the tm input.
        for i in range(NI):
            pairs = [(j, hf) for j in range(i + 1) for hf in range(2)]
            nt = len(pairs)
            SC = pool.tile([NH, 8, S], BF16, name="sct", tag="sct", bufs=1)
            for tk, (j, hf) in enumerate(pairs):
                pr = pool.tile([128, KT, S], BF16, name="prt", tag="prt", bufs=1)
                for k in range(KT):
                    nc.vector.tensor_mul(pr[:, k, :], QT[k][:, i, :],
                                         KTa[k][:, hf, j, :])
                pp = psum.tile([NH, S], F32, name="ppt", tag="ps_sm", bufs=2)
                for k in range(KT):
                    nc.tensor.matmul(pp[:], HeadInd[k], pr[:, k, :],
                                     start=(k == 0), stop=(k == KT - 1))
                nc.scalar.activation(SC[:, tk, :], pp[:], AF.Exp, scale=0.125)
                if j == i and hf == 1:
                    nc.vector.tensor_scalar_mul(SC[:, tk, :], SC[:, tk, :],
                                                tm_sb[:, i:i + 1])
            rd = spool.tile([NH, S], F32, name="rd", tag="rd", bufs=1)
            nc.vector.tensor_reduce(rd[:],
                                    SC[:, 0:nt, :].rearrange("p t s -> p s t"),
                                    axis=mybir.AxisListType.X, op=ALU.add)
            nc.scalar.activation(rd[:], rd[:], AF.Ln)
            nc.scalar.activation(rd[:], rd[:], AF.Exp, scale=-1.0)
            for tk in range(nt):
                nc.vector.tensor_mul(SC[:, tk, :], SC[:, tk, :], rd[:])
            for k in range(KT):
                acc = psum.tile([128, S], F32, name="acct", tag="ps_att", bufs=2)
                for tk, (j, hf) in enumerate(pairs):
                    pb = psum.tile([128, S], F32, name="pbt", tag="ps_bc", bufs=1)
                    nc.tensor.matmul(pb[:], HeadSel[k], SC[:, tk, :],
                                     start=True, stop=True)
                    if tk == 0:
                        nc.vector.tensor_mul(acc[:], pb[:], VTa[k][:, hf, j, :])
                    else:
                        tmpv = pool.tile([128, S], F32, name="tmpv", tag="tmpv",
                                         bufs=1)
                        nc.vector.tensor_mul(tmpv[:], pb[:], VTa[k][:, hf, j, :])
                        nc.vector.tensor_add(acc[:], acc[:], tmpv[:])
                nc.vector.tensor_copy(AO[k][:, i, :], acc[:])

    # ---------------- transformer block ----------------
    def block(bi, kind, mode):
        w = g["blk_w"][bi]
        cur["wqkv_fn"] = stream_w(w["wqkv"], KT, tag="wms", bufs=4)
        cur["bqkv"] = load_cols(f"bqkv_sb{bi}", w["bqkv"], 12, tag="c_bqkv", bufs=4)
        layernorm()
        if kind == "space":
            bvr = spool.tile([1, E], F32, name="bvr", tag="rowtmp", bufs=1)
            nc.sync.dma_start(bvr[:], w["bqv"][:])
            bqvb = spool.tile([1, E], BF16, name="bqvb", tag="bqvb", bufs=1)
            nc.vector.tensor_copy(bqvb[:], bvr[:])
            cur["bqvb"] = bqvb

            def qk_out(mt, n0, nw, ps):
                if mt < KT:
                    dst = flat(QT[mt])[:, n0:n0 + nw]
                else:
                    dst = flat(KTa[mt - KT][:, 0, :, :])[:, n0:n0 + nw]
                nc.scalar.activation(dst, ps[:, :nw], AF.Identity,
                                     bias=cur["bqkv"][:, mt:mt + 1])

            wv = wpool.tile([128, KT, KT, 128], BF16, name="wv", tag="wv",
                            bufs=2)
            for vmt in range(KT):
                nc.sync.dma_start(
                    wv[:, :, vmt, :],
                    w["wqkv"][:, (8 + vmt) * KT * 128:(9 + vmt) * KT * 128]
                    .rearrange("p (k m) -> p k m", k=KT))
            cur["wv"] = wv
            linearT(cur["wqkv_fn"], KT, 2 * KT,
                    lambda k, n0, nw: flat(hT[k])[:, n0:n0 + nw], qk_out)
            space_attention(mode)
        else:
            time_attention()
        bproj_sb = load_cols(f"bproj_sb{bi}", w["bproj"], KT, tag="c_b", bufs=4)
        linearT(stream_w(w["wproj"], KT, tag="wms", bufs=4), KT, KT,
                lambda k, n0, nw: flat(AO[k])[:, n0:n0 + nw],
                lambda mt, n0, nw, ps: resid_add(mt, n0, nw, ps,
                                                 bproj_sb[:, mt:mt + 1]))
        layernorm()
        bfc1_sb = load_cols(f"bfc1_sb{bi}", w["bfc1"], MLP // 128, tag="c_fc1", bufs=4)
        wfc2_fn = stream_w(w["wfc2"], MLP // 128, tag="w2s", bufs=2)
        bfc2_sb = load_cols(f"bfc2_sb{bi}", w["bfc2"], KT, tag="c_b", bufs=4)
        for (n0, nw) in CHUNKS:
            gt = pool.tile([128, MLP // 128, 512], BF16, name="gt", tag="gt",
                           bufs=1)
            for mt in range(MLP // 128):
                w1 = wpool.tile([128, KT, 128], BF16, name="w1", tag="w1s",
                                bufs=4)
                nc.sync.dma_start(
                    w1[:], w["wfc1"][:, mt * KT * 128:(mt + 1) * KT * 128]
                    .rearrange("p (k m) -> p k m", k=KT))
                ps = psum.tile([128, 512], F32, name="f1p", tag="ps_mm", bufs=3)
                for k in range(KT):
                    nc.tensor.matmul(ps[:, :nw], w1[:, k, :],
                                     flat(hT[k])[:, n0:n0 + nw],
                                     start=(k == 0), stop=(k == KT - 1))
                nc.scalar.activation(gt[:, mt, :nw], ps[:, :nw],
                                     AF.Gelu_apprx_tanh,
                                     bias=bfc1_sb[:, mt:mt + 1])
            for mo in range(KT):
                w2 = wfc2_fn(mo)
                ps = psum.tile([128, 512], F32, name="f2p", tag="ps_mm", bufs=3)
                for k in range(MLP // 128):
                    nc.tensor.matmul(ps[:, :nw], w2[:, k, :], gt[:, k, :nw],
                                     start=(k == 0), stop=(k == MLP // 128 - 1))
                resid_add(mo, n0, nw, ps, bfc2_sb[:, mo:mo + 1])

    # encoder
    for bi in range(3):
        block(bi, "space", "enc")
    block(3, "time", None)

    # to_latent + tanh -> z
    zin = pool.tile([128, KT, NI, NL], BF16, name="zin", tag="bigstage")
    for k in range(KT):
        nc.vector.tensor_copy(zin[:, k, :, :], XT[k][:, :, 0:NL])
    wtl_sb = stream_w(g["wtl"], KT, tag="wms", bufs=4)(0)
    zps = psum.tile([128, 512], F32, name="zps", tag="ps_mm", bufs=3)
    for k in range(KT):
        nc.tensor.matmul(zps[:], wtl_sb[:, k, :],
                         zin[:, k].rearrange("p i s -> p (i s)"),
                         start=(k == 0), stop=(k == KT - 1))
    zT_sb = pool.tile([128, NI * NL], F32, name="zT_sb", tag="zT_sb")
    nc.scalar.activation(zT_sb[:], zps[:], AF.Tanh, bias=btl_sb[:, 0:1])
    nc.sync.dma_start(g["z_T"][:], zT_sb[:])

    if STAGE == "enc":
        ctx.close()
        return

    # from_latent -> XT latent cols; decoder tokens -> patch cols
    zb = pool.tile([128, NI * NL], BF16, name="zb", tag="zb")
    nc.vector.tensor_copy(zb[:], zT_sb[:])
    wfl_fn = stream_w(g["wfl"], 1, tag="wms", bufs=4)
    for mt in range(KT):
        wfl_t = wfl_fn(mt)
        ps = psum.tile([128, 512], F32, name="flp", tag="ps_mm", bufs=3)
        nc.tensor.matmul(ps[:], wfl_t[:, 0, :], zb[:],
                         start=True, stop=True)
        nc.scalar.activation(
            XT[mt][:, :, 0:NL], ps[:].rearrange("p (i s) -> p i s", i=NI),
            AF.Identity, bias=bfl_sb[:, mt:mt + 1])
    for k in range(KT):
        for i in range(NI):
            nc.sync.dma_start(XT[k][:, i, NL:],
                              g["decT"][k * 128:(k + 1) * 128, :])

    # decoder
    for bi in range(3):
        block(4 + bi, "space", "dec")
    block(7, "time", None)

    # to_pixels on patch tokens
    hP = pool.tile([128, KT, NI, N], BF16, name="hP", tag="bigstage")
    for k in range(KT):
        nc.vector.tensor_copy(hP[:, k, :, :], XT[k][:, :, NL:])
    wtp_fn = stream_w(g["wtp"], KT, tag="wms", bufs=4)

    def tp_out(mt, n0, nw, ps):
        stg = pool.tile([128, N], F32, name="tpst", tag="stg_f32", bufs=3)
        nc.scalar.activation(stg[:], ps[:, :nw], AF.Identity,
                             bias=btp_sb[:, mt:mt + 1])
        nc.sync.dma_start(g["rp_T"][mt * 128:(mt + 1) * 128, n0:n0 + nw],
                          stg[:])

    linearT(wtp_fn, KT, PD // 128,
            lambda k, n0, nw: hP[:, k, n0 // N, :], tp_out,
            n_slices=img_slices)
    ctx.close()


# ---------------- host side ----------------
_prog_cache = {}


def _get_prog():
    if "nc" not in _prog_cache:
        _prog_cache["nc"] = build_program()
    return _prog_cache["nc"]


def _host_mask(noise, mask_ratios):
    ids_shuffle = np.argsort(noise, axis=2, kind="stable")
    ids_unshuffle = np.argsort(ids_shuffle, axis=2, kind="stable")
    len_keep = (np.float32(N) * (np.float32(1.0) -
                                 mask_ratios.astype(np.float32) *
                                 np.float32(0.9))).astype(np.int32)
    vis = (np.arange(N)[None, None, :] < len_keep[:, None, None]).astype(np.float32)
    return np.take_along_axis(np.broadcast_to(vis, (B, T, N)).copy(),
                              ids_unshuffle, axis=2)


def kernel(x, noise, mask_ratios, params):
    x = np.asarray(x, np.float32)
    noise = np.asarray(noise, np.float32)
    mask_ratios = np.asarray(mask_ratios, np.float32)
    p = params

    mask = _host_mask(noise, mask_ratios)
    xp = x.reshape(B, T, C, G, PP, G, PP).transpose(0, 1, 3, 5, 4, 6, 2) \
          .reshape(B, T, N, PD)

    hind = np.zeros((128, KT, NH), np.float32)
    hsel = np.zeros((8, KT, 128), np.float32)
    for k in range(KT):
        hind[0:64, k, 2 * k] = 1.0
        hind[64:128, k, 2 * k + 1] = 1.0
        hsel[2 * k, k, 0:64] = 1.0
        hsel[2 * k + 1, k, 64:128] = 1.0
    shared = {
        "hind": _bf(hind.reshape(128, KT * NH)),
        "hsel": _bf(hsel.reshape(8, KT * 128)),
        "wpe": _pack_w(p["patch_embed"]["w"]), "bpe": _col(p["patch_embed"]["b"]),
        "mtok": _col(p["mask_token"]),
        "latT": _f32(np.asarray(p["latent_tokens"]).T),
        "decT": _f32(np.asarray(p["decoder_tokens"]).T),
        "wtl": _pack_w(p["to_latent"]["w"]), "btl": _col(p["to_latent"]["b"]),
        "wfl": _pack_w(p["from_latent"]["w"]), "bfl": _col(p["from_latent"]["b"]),
        "wtp": _pack_w(p["to_pixels"]["w"]), "btp": _col(p["to_pixels"]["b"]),
    }
    allblk = list(p["enc_blocks"]) + list(p["dec_blocks"])
    for bi in range(8):
        blk = allblk[bi]
        l1s, l1b = _f32(blk["ln1_s"]), _f32(blk["ln1_b"])
        l2s, l2b = _f32(blk["ln2_s"]), _f32(blk["ln2_b"])
        wqkv = _f32(blk["qkv"]["w"])
        wqkv_s = l1s[:, None] * wqkv
        bqkv_f = _f32(blk["qkv"]["b"]) + l1b @ wqkv
        wfc1 = _f32(blk["fc1"]["w"])
        wfc1_s = l2s[:, None] * wfc1
        bfc1_f = _f32(blk["fc1"]["b"]) + l2b @ wfc1
        shared.update({
            f"wqkv{bi}": _pack_w(wqkv_s), f"bqkv{bi}": _col(bqkv_f),
            f"bqv{bi}": bqkv_f[2 * E:3 * E].reshape(1, E).copy(),
            f"wproj{bi}": _pack_w(blk["proj"]["w"]), f"bproj{bi}": _col(blk["proj"]["b"]),
            f"wfc1{bi}": _pack_w(wfc1_s), f"bfc1{bi}": _col(bfc1_f),
            f"wfc2{bi}": _pack_w(blk["fc2"]["w"]), f"bfc2{bi}": _col(blk["fc2"]["b"]),
            f"ln1s{bi}": _col(l1s), f"ln1b{bi}": _col(l1b),
            f"ln2s{bi}": _col(l2s), f"ln2b{bi}": _col(l2b),
        })

    core_t = [[2 * i + (k % 2) for i in range(NI)] for k in range(NCORES)]
    in_maps = []
    for k in range(NCORES):
        b, pbit = k // 2, k % 2
        ts = core_t[k]
        xpc = xp[b, ts]                       # [NI, N, PD]
        m = dict(shared)
        xpt = xpc.transpose(2, 0, 1).reshape(PD // 128, 128, NI * N)
        m["xpT"] = _bf(xpt.transpose(1, 0, 2).reshape(128, (PD // 128) * NI * N))
        m["maskrow"] = _f32(mask[b, ts].reshape(1, NI * N))
        m["tm"] = np.full((8, NI), 0.0 if pbit == 0 else 1.0, np.float32)
        in_maps.append(m)

    nc = _get_prog()
    res = run_bass_kernel_spmd(nc, in_maps, list(range(NCORES)))

    patches = np.zeros((B, T, N, E), np.float32)
    z = np.zeros((B, T, NL, LD), np.float32)
    rp = np.zeros((B, T, N, PD), np.float32)
    for k in range(NCORES):
        b, ts = k // 2, core_t[k]
        r = res.results[k]
        patches[b, ts] = r["patches_T"].reshape(E, NI, N).transpose(1, 2, 0)
        z[b, ts] = r["z_T"].reshape(LD, NI, NL).transpose(1, 2, 0)
        rp[b, ts] = r["rp_T"].reshape(PD, NI, N).transpose(1, 2, 0)

    recon = rp.reshape(B, T, G, G, PP, PP, C).transpose(0, 1, 6, 2, 4, 3, 5) \
              .reshape(B, T, C, IMG, IMG)
    return z, recon, mask.astype(np.float32), patches
